# revision 1
# baseline (speedup 1.0000x reference)
"""Trainium2 Bass kernel for a conditional set encoder (ragged sequences).

Sharding: data-parallel over the task/batch dim B=64 across 8 NeuronCores
(8 tasks per core); all weights replicated.

On-chip layout: activations are kept feature-major ("T layout", [feat, row])
so every feature contraction has the contracted dim on partitions and the
weights [K, N] act directly as the stationary (lhsT) operand.

Attention is computed per head in [k, q] layout:
  E[k, q] = exp(scale * k.q + mask_bias[k])           (mask folds into ACT bias)
  oT[d, q], s[q] = (V_ext[k, d|1])^T @ E[k, q]        (ones column -> softmax sums)
then o is normalized by 1/s broadcast across partitions with a tiny K=8
outer-product matmul.

NaN-marked (padded) context rows are zeroed with a bitwise trick:
  valid = (x == x); Cz = C & (valid * -1 as int32)    (kills NaNs exactly)
Masked rows then flow through the MLP as exact zeros; attention gives them
exactly zero weight via exp(-1e30) = 0, matching the reference.
"""

import math
from contextlib import ExitStack

import numpy as np

import concourse.bacc as bacc
import concourse.bass as bass  # noqa: F401  (bass types via bacc)
import concourse.tile as tile
from concourse import mybir
from concourse.alu_op_type import AluOpType
from concourse.bass_utils import run_bass_kernel_spmd
from concourse.masks import make_identity

# Problem dims (hardcoded per spec)
B, NC, ND = 64, 1024, 1024
DX, DY, DH = 64, 64, 512
NH, HD = 8, 64
NCORES = 8
TPC = B // NCORES  # tasks per core

F32 = mybir.dt.float32
F32R = mybir.dt.float32r
I32 = mybir.dt.int32
AF = mybir.ActivationFunctionType

SCALE = 1.0 / math.sqrt(DH)
NEGBIG = -1.0e30

MM_DT = F32R  # matmul streaming dtype: float32r is full-rate at N>=256


def _ceil(a, b):
    return (a + b - 1) // b


def build_nc(n_tasks=TPC, mm_dt=MM_DT):
    nc = bacc.Bacc(None, target_bir_lowering=False, dynamic_dma_scratch_size=256)

    def mm(out, lhsT, rhs, **kw):
        nc.tensor.matmul(out, lhsT.bitcast(mm_dt), rhs.bitcast(mm_dt), **kw)

    def r(ap):
        # write-side cast: producers of matmul inputs must round to the
        # matmul dtype (walrus verifies fp32r producer/consumer pairing)
        return ap.bitcast(mm_dt)


    # ---- DRAM I/O ----
    C_d = nc.dram_tensor("C", [n_tasks, NC, DX + DY], F32, kind="ExternalInput")
    XC_d = nc.dram_tensor("X_C", [n_tasks, NC, DX], F32, kind="ExternalInput")
    XD_d = nc.dram_tensor("X_D", [n_tasks, ND, DX], F32, kind="ExternalInput")
    W0_d = nc.dram_tensor("W0", [DX + DY, DH], F32, kind="ExternalInput")
    b0_d = nc.dram_tensor("b0", [DH], F32, kind="ExternalInput")
    Wm_d = nc.dram_tensor("Wm", [2, DH, DH], F32, kind="ExternalInput")
    bm_d = nc.dram_tensor("bm", [2, DH], F32, kind="ExternalInput")
    Wl_d = nc.dram_tensor("Wl", [DH, DH], F32, kind="ExternalInput")
    bl_d = nc.dram_tensor("bl", [DH], F32, kind="ExternalInput")
    temb_d = nc.dram_tensor("task_emb", [DH], F32, kind="ExternalInput")
    Wq0_d = nc.dram_tensor("Wq0", [DX, DH], F32, kind="ExternalInput")
    Wk0_d = nc.dram_tensor("Wk0", [DX, DH], F32, kind="ExternalInput")
    Wv0_d = nc.dram_tensor("Wv0", [DH, DH], F32, kind="ExternalInput")
    Wo0_d = nc.dram_tensor("Wo0", [DH, DH], F32, kind="ExternalInput")
    bo0_d = nc.dram_tensor("bo0", [DH], F32, kind="ExternalInput")
    Wq1_d = nc.dram_tensor("Wq1", [DH, DH], F32, kind="ExternalInput")
    Wk1_d = nc.dram_tensor("Wk1", [DX, DH], F32, kind="ExternalInput")
    Wv1_d = nc.dram_tensor("Wv1", [DH, DH], F32, kind="ExternalInput")
    Wo1_d = nc.dram_tensor("Wo1", [DH, DH], F32, kind="ExternalInput")
    bo1_d = nc.dram_tensor("bo1", [DH], F32, kind="ExternalInput")
    OUT_d = nc.dram_tensor("OUT", [n_tasks, ND, DH], F32, kind="ExternalOutput")

    KC = 8   # 128-row chunks of the 1024 context/target rows
    CH = 4   # 128-feature chunks of DH

    with tile.TileContext(nc) as tc, ExitStack() as ctx, \
            nc.allow_low_precision(reason="fp32r write-side rounding is intentional"):
        wp = ctx.enter_context(tc.tile_pool(name="wp", bufs=1))
        czp = ctx.enter_context(tc.tile_pool(name="czp", bufs=1))
        xp = ctx.enter_context(tc.tile_pool(name="xp", bufs=1))
        craw = ctx.enter_context(tc.tile_pool(name="craw", bufs=2))
        maskp = ctx.enter_context(tc.tile_pool(name="maskp", bufs=2))
        hp = ctx.enter_context(tc.tile_pool(name="hp", bufs=2))
        dp = ctx.enter_context(tc.tile_pool(name="dp", bufs=1))
        vexp = ctx.enter_context(tc.tile_pool(name="vexp", bufs=16))
        bigp = ctx.enter_context(tc.tile_pool(name="bigp", bufs=3))
        ep = ctx.enter_context(tc.tile_pool(name="ep", bufs=2))
        ftp = ctx.enter_context(tc.tile_pool(name="ftp", bufs=2))
        sp = ctx.enter_context(tc.tile_pool(name="sp", bufs=1))
        urp = ctx.enter_context(tc.tile_pool(name="urp", bufs=2))
        stp = ctx.enter_context(tc.tile_pool(name="stp", bufs=4))
        pps = ctx.enter_context(tc.tile_pool(name="pps", bufs=6, space="PSUM"))

        def psum():
            ps = pps.tile([128, 512], F32, name="ps", tag="ps")
            return ps

        # ---- constants & weights (loaded once, reused by all tasks) ----
        ident = wp.tile([128, 128], F32, name="ident")
        make_identity(nc, ident)

        ones1f = wp.tile([1, 128], F32, name="ones1f")
        nc.vector.memset(ones1f, 1.0)
        ones1 = wp.tile([1, 128], F32, name="ones1")
        nc.vector.tensor_copy(r(ones1), ones1f)
        # shared rounded ones block for the v_ext ones columns
        vonesf = wp.tile([128, 8], F32, name="vonesf")
        nc.vector.memset(vonesf, 1.0)
        vones = wp.tile([128, 8], F32, name="vones")
        nc.vector.tensor_copy(r(vones), vonesf)

        # b8c[c][k, p] = 1 iff k == 2c + p//64 : broadcasts S rows (2c,2c+1)
        # over the 128 partitions of feature-chunk c.
        bdiff = wp.tile([8, 128], I32, name="bdiff")
        # bdiff[k, p] = k - p//64
        nc.gpsimd.iota(bdiff, pattern=[[-1, 2], [0, 64]], channel_multiplier=1)
        bdifff = wp.tile([8, 128], F32, name="bdifff")
        nc.vector.tensor_copy(bdifff, bdiff)
        b8 = []
        for c in range(CH):
            b8c = wp.tile([8, 128], F32, name=f"b8_{c}", tag=f"b8_{c}")
            nc.vector.tensor_scalar(out=r(b8c), in0=bdifff, scalar1=float(2 * c),
                                    scalar2=None, op0=AluOpType.is_equal)
            b8.append(b8c)

        def load_w_chunked(name, src_ap):
            # [DH, DH] weight -> [128, CH, DH]: partition p + chunk c = row c*128+p
            t = wp.tile([128, CH, DH], F32, name=name, tag=name)
            nc.sync.dma_start(
                out=r(t), in_=src_ap.rearrange("(c p) n -> p c n", p=128).bitcast(mm_dt))
            return t

        def load_col(name, src_ap):
            # [DH] vector -> [128, CH] column tiles
            t = wp.tile([128, CH], F32, name=name, tag=name)
            nc.sync.dma_start(out=t, in_=src_ap.rearrange("(c p) -> p c", p=128))
            return t

        W0s = wp.tile([128, DH], F32, name="W0s")
        nc.sync.dma_start(out=r(W0s), in_=W0_d[:, :].bitcast(mm_dt))
        Wm0s = load_w_chunked("Wm0s", Wm_d[0])
        Wm1s = load_w_chunked("Wm1s", Wm_d[1])
        Wls = load_w_chunked("Wls", Wl_d[:, :])
        Wv0s = load_w_chunked("Wv0s", Wv0_d[:, :])
        Wo0s = load_w_chunked("Wo0s", Wo0_d[:, :])
        Wq1s = load_w_chunked("Wq1s", Wq1_d[:, :])
        Wv1s = load_w_chunked("Wv1s", Wv1_d[:, :])
        Wo1s = load_w_chunked("Wo1s", Wo1_d[:, :])

        Wq0s = wp.tile([64, DH], F32, name="Wq0s")
        nc.sync.dma_start(out=r(Wq0s), in_=Wq0_d[:, :].bitcast(mm_dt))
        Wk0s = wp.tile([64, DH], F32, name="Wk0s")
        nc.sync.dma_start(out=r(Wk0s), in_=Wk0_d[:, :].bitcast(mm_dt))
        Wk1s = wp.tile([64, DH], F32, name="Wk1s")
        nc.sync.dma_start(out=r(Wk1s), in_=Wk1_d[:, :].bitcast(mm_dt))

        b0c = load_col("b0c", b0_d[:])
        bm0c = load_col("bm0c", bm_d[0])
        bm1c = load_col("bm1c", bm_d[1])
        blc = load_col("blc", bl_d[:])
        tembc = load_col("tembc", temb_d[:])
        bo0c = load_col("bo0c", bo0_d[:])
        # d-layer bias: bl + task_emb fused
        dbc = wp.tile([128, CH], F32, name="dbc")
        nc.vector.tensor_add(dbc, blc, tembc)
        bo1r = wp.tile([1, DH], F32, name="bo1r")
        nc.sync.dma_start(out=r(bo1r),
                          in_=bo1_d[:].rearrange("(a n) -> a n", a=1).bitcast(mm_dt))

        def proj64(Ws, xT, outT):
            """outT[feat, m] = (x @ W)^T for 64-dim input x (xT: [64, 1024])."""
            for c in range(CH):
                for mh in range(2):
                    ms = slice(mh * 512, mh * 512 + 512)
                    ps = psum()
                    mm(ps, Ws[:, c * 128 : (c + 1) * 128], xT[:, ms],
                       start=True, stop=True)
                    nc.any.tensor_copy(r(outT[:, c, ms]), ps)

        def attn_block(qT, kT, ve_list, maskb, uT):
            """uT = qT + softmax(scale*q.k + mask) @ v   (all in T layout)."""
            S = sp.tile([8, ND], F32, name="S", tag="S")
            for hp2 in range(NH // 2):
                he, ho = 2 * hp2, 2 * hp2 + 1
                c = hp2  # feature chunk holding this head pair
                for qc in range(2):
                    qs = slice(qc * 512, qc * 512 + 512)
                    orwE = psum()
                    orwO = psum()
                    for kc in range(KC):
                        ks = slice(kc * 128, kc * 128 + 128)
                        lgE = psum()
                        lgO = psum()
                        # [k,q] logits; even head on PE rows 0-63, odd on 64-127
                        mm(lgE, kT[0:64, c, ks], qT[0:64, c, qs],
                           start=True, stop=True)
                        mm(lgO, kT[64:128, c, ks], qT[64:128, c, qs],
                           start=True, stop=True)
                        EE = ep.tile([128, 512], F32, name="EE", tag="E")
                        EO = ep.tile([128, 512], F32, name="EO", tag="E")
                        nc.scalar.activation(r(EE), lgE, AF.Exp,
                                             bias=maskb[:, kc : kc + 1], scale=SCALE)
                        nc.scalar.activation(r(EO), lgO, AF.Exp,
                                             bias=maskb[:, kc : kc + 1], scale=SCALE)
                        mm(orwE[0:65, :], ve_list[kc][:, he, :], EE,
                           start=(kc == 0), stop=(kc == KC - 1))
                        mm(orwO[0:65, :], ve_list[kc][:, ho, :], EO,
                           start=(kc == 0), stop=(kc == KC - 1))
                    # stash unnormalized oT and the softmax sums; sums hop
                    # through an SBUF stage row (engine APs can only start at
                    # 32-aligned partitions; DMA can write any partition).
                    nc.any.tensor_copy(r(uT[0:64, c, qs]), orwE[0:64, :])
                    nc.any.tensor_copy(r(uT[64:128, c, qs]), orwO[0:64, :])
                    stE = stp.tile([1, 512], F32, name="stE", tag="st")
                    stO = stp.tile([1, 512], F32, name="stO", tag="st")
                    nc.vector.tensor_copy(r(stE), orwE[64:65, :])
                    nc.vector.tensor_copy(r(stO), orwO[64:65, :])
                    nc.sync.dma_start(out=r(S[he : he + 1, qs]), in_=r(stE))
                    nc.sync.dma_start(out=r(S[ho : ho + 1, qs]), in_=r(stO))
            nc.vector.reciprocal(r(S), S)
            # normalize by 1/s (partition-broadcast via K=8 outer product) + resid
            for c in range(CH):
                for mh in range(2):
                    ms = slice(mh * 512, mh * 512 + 512)
                    bc = psum()
                    mm(bc, b8[c], S[:, ms], start=True, stop=True)
                    nc.vector.tensor_mul(r(uT[:, c, ms]), uT[:, c, ms], bc)
                nc.vector.tensor_add(r(uT[:, c, :]), uT[:, c, :], qT[:, c, :])

        # =================== per-task pipeline ===================
        for t in range(n_tasks):
            # ---- stage A: load, mask, transpose inputs ----
            validc = maskp.tile([128, KC], F32, name="validc", tag="validc")
            maskb = maskp.tile([128, KC], F32, name="maskb", tag="maskb")
            czT = czp.tile([128, NC], F32, name="czT", tag="czT")
            xcT = xp.tile([64, NC], F32, name="xcT", tag="xcT")
            xdT = xp.tile([64, NC], F32, name="xdT", tag="xdT")
            for rc in range(KC):
                rs = slice(rc * 128, rc * 128 + 128)
                c_raw = craw.tile([128, 128], F32, name="c_raw", tag="c_raw")
                nc.sync.dma_start(out=c_raw, in_=C_d[t, rs, :])
                lastc = c_raw[:, 127:128]
                nc.vector.tensor_tensor(out=validc[:, rc : rc + 1], in0=lastc,
                                        in1=lastc, op=AluOpType.is_equal)
                nc.vector.tensor_scalar(out=maskb[:, rc : rc + 1],
                                        in0=validc[:, rc : rc + 1],
                                        scalar1=1.0e30, scalar2=NEGBIG,
                                        op0=AluOpType.mult, op1=AluOpType.add)
                vfull = craw.tile([128, 128], I32, name="vfull", tag="vfull")
                nc.vector.tensor_tensor(out=vfull, in0=c_raw, in1=c_raw,
                                        op=AluOpType.is_equal)
                cz = craw.tile([128, 128], F32, name="cz", tag="cz")
                nc.vector.memset(cz, 0.0)
                nc.vector.copy_predicated(out=cz, mask=vfull, data=c_raw)
                ps = psum()
                nc.tensor.transpose(ps[:, 0:128], cz, ident)
                nc.any.tensor_copy(r(czT[:, rs]), ps[:, 0:128])

                xc_raw = craw.tile([128, 64], F32, name="xc_raw", tag="xc_raw")
                nc.sync.dma_start(out=xc_raw, in_=XC_d[t, rs, :])
                ps2 = psum()
                nc.tensor.transpose(ps2[0:64, 0:128], xc_raw, ident)
                nc.any.tensor_copy(r(xcT[:, rs]), ps2[0:64, 0:128])

                xd_raw = craw.tile([128, 64], F32, name="xd_raw", tag="xd_raw")
                nc.sync.dma_start(out=xd_raw, in_=XD_d[t, rs, :])
                ps3 = psum()
                nc.tensor.transpose(ps3[0:64, 0:128], xd_raw, ident)
                nc.any.tensor_copy(r(xdT[:, rs]), ps3[0:64, 0:128])

            # ---- MLP + V projections, per 512-row half ----
            v0e = [None] * KC
            v1e = [None] * KC
            for mh in range(2):
                ms = slice(mh * 512, mh * 512 + 512)
                h1 = hp.tile([128, CH, 512], F32, name="h1", tag="h")
                for c in range(CH):
                    ps = psum()
                    mm(ps, W0s[:, c * 128 : (c + 1) * 128], czT[:, ms],
                       start=True, stop=True)
                    nc.scalar.activation(r(h1[:, c, :]), ps, AF.Relu,
                                         bias=b0c[:, c : c + 1])
                h2 = hp.tile([128, CH, 512], F32, name="h2", tag="h")
                for c in range(CH):
                    ps = psum()
                    for kc in range(CH):
                        mm(ps, Wm0s[:, kc, c * 128 : (c + 1) * 128], h1[:, kc, :],
                           start=(kc == 0), stop=(kc == CH - 1))
                    nc.scalar.activation(r(h2[:, c, :]), ps, AF.Relu,
                                         bias=bm0c[:, c : c + 1])
                h3 = hp.tile([128, CH, 512], F32, name="h3", tag="h")
                for c in range(CH):
                    ps = psum()
                    for kc in range(CH):
                        mm(ps, Wm1s[:, kc, c * 128 : (c + 1) * 128], h2[:, kc, :],
                           start=(kc == 0), stop=(kc == CH - 1))
                    nc.scalar.activation(r(h3[:, c, :]), ps, AF.Relu,
                                         bias=bm1c[:, c : c + 1])
                d_t = dp.tile([128, CH, 512], F32, name="d_t", tag="d")
                for c in range(CH):
                    ps = psum()
                    for kc in range(CH):
                        mm(ps, Wls[:, kc, c * 128 : (c + 1) * 128], h3[:, kc, :],
                           start=(kc == 0), stop=(kc == CH - 1))
                    nc.scalar.activation(r(d_t[:, c, :]), ps, AF.Identity,
                                         bias=dbc[:, c : c + 1])
                # v0/v1 for this half's row chunks (row-major + ones column)
                for j in range(4):
                    rc = mh * 4 + j
                    js = slice(j * 128, j * 128 + 128)
                    for vlist, Wvs in ((v0e, Wv0s), (v1e, Wv1s)):
                        ps = psum()
                        for kc in range(CH):
                            mm(ps, d_t[:, kc, js], Wvs[:, kc, :],
                               start=(kc == 0), stop=(kc == CH - 1))
                        ve = vexp.tile([128, NH, HD + 1], F32, name="ve", tag="ve")
                        nc.vector.tensor_copy(r(ve[:, :, HD : HD + 1]),
                              vones.rearrange("p (h a) -> p h a", a=1))
                        nc.any.tensor_copy(
                            r(ve[:, :, 0:HD]),
                            ps.rearrange("p (h e) -> p h e", h=NH))
                        vlist[rc] = ve

            # ---- projections for block 0 ----
            k0T = bigp.tile([128, CH, ND], F32, name="k0T", tag="big")
            proj64(Wk0s, xcT, k0T)
            q0T = bigp.tile([128, CH, ND], F32, name="q0T", tag="big")
            proj64(Wq0s, xdT, q0T)

            # ---- block 0 attention (+ residual into uT) ----
            uT = bigp.tile([128, CH, ND], F32, name="uT", tag="big")
            attn_block(q0T, k0T, v0e, maskb, uT)

            # k1 projection (placed here: fills PE while block-0 ACT drains)
            k1T = bigp.tile([128, CH, ND], F32, name="k1T", tag="big")
            proj64(Wk1s, xcT, k1T)

            # ---- block 0 FFN: u2 = u + relu(u @ Wo0 + bo0) ----
            u2T = bigp.tile([128, CH, ND], F32, name="u2T", tag="big")
            for c in range(CH):
                for mh in range(2):
                    ms = slice(mh * 512, mh * 512 + 512)
                    ps = psum()
                    for kc in range(CH):
                        mm(ps, Wo0s[:, kc, c * 128 : (c + 1) * 128],
                           uT[:, kc, ms], start=(kc == 0), stop=(kc == CH - 1))
                    ft = ftp.tile([128, 512], F32, name="ft", tag="ft")
                    nc.scalar.activation(ft, ps, AF.Relu, bias=bo0c[:, c : c + 1])
                    nc.vector.tensor_add(r(u2T[:, c, ms]), uT[:, c, ms], ft)

            # ---- q1 projection ----
            q1T = bigp.tile([128, CH, ND], F32, name="q1T", tag="big")
            for c in range(CH):
                for mh in range(2):
                    ms = slice(mh * 512, mh * 512 + 512)
                    ps = psum()
                    for kc in range(CH):
                        mm(ps, Wq1s[:, kc, c * 128 : (c + 1) * 128],
                           u2T[:, kc, ms], start=(kc == 0), stop=(kc == CH - 1))
                    nc.any.tensor_copy(r(q1T[:, c, ms]), ps)

            # ---- block 1 attention ----
            uT1 = bigp.tile([128, CH, ND], F32, name="uT1", tag="big")
            attn_block(q1T, k1T, v1e, maskb, uT1)

            # ---- block 1 FFN in row-major + output ----
            for j in range(KC):
                js = slice(j * 128, j * 128 + 128)
                fp = psum()
                # bias via K=1 outer product, then accumulate u @ Wo1
                mm(fp, ones1, bo1r, start=True, stop=False)
                for kc in range(CH):
                    mm(fp, uT1[:, kc, js], Wo1s[:, kc, :],
                       start=False, stop=(kc == CH - 1))
                fr = ftp.tile([128, 512], F32, name="fr", tag="ft")
                nc.scalar.activation(fr, fp, AF.Relu)
                ur = urp.tile([128, DH], F32, name="ur", tag="ur")
                for c in range(CH):
                    cs = slice(c * 128, c * 128 + 128)
                    tp = psum()
                    nc.tensor.transpose(tp[:, 0:128], uT1[:, c, js], ident)
                    nc.vector.tensor_add(ur[:, cs], fr[:, cs], tp[:, 0:128])
                nc.sync.dma_start(out=OUT_d[t, js, :], in_=ur)

    nc.compile()
    return nc


_NC_CACHE = {}


def _get_nc():
    key = (TPC, MM_DT)
    if key not in _NC_CACHE:
        _NC_CACHE[key] = build_nc(TPC, MM_DT)
    return _NC_CACHE[key]


def _as_f32(x):
    return np.ascontiguousarray(np.asarray(x, dtype=np.float32))


def run(inputs, trace=False, **kw):
    nc = _get_nc()
    w_names = ["W0", "b0", "Wm", "bm", "Wl", "bl", "task_emb",
               "Wq0", "Wk0", "Wv0", "Wo0", "bo0",
               "Wq1", "Wk1", "Wv1", "Wo1", "bo1"]
    weights = {k: _as_f32(inputs[k]) for k in w_names}
    C = _as_f32(inputs["C"])
    XC = _as_f32(inputs["X_C"])
    XD = _as_f32(inputs["X_D"])
    in_maps = []
    for i in range(NCORES):
        s = slice(i * TPC, (i + 1) * TPC)
        m = dict(weights)
        m["C"] = C[s]
        m["X_C"] = XC[s]
        m["X_D"] = XD[s]
        in_maps.append(m)
    res = run_bass_kernel_spmd(nc, in_maps, core_ids=list(range(NCORES)),
                               trace=trace, **kw)
    out = np.concatenate([res.results[i]["OUT"] for i in range(NCORES)], axis=0)
    return out, res


def kernel(**inputs) -> np.ndarray:
    out, _ = run(inputs, trace=False)
    return out



# revision 4
# speedup vs baseline: 1.7002x; 1.7002x over previous
"""Trainium2 Bass kernel for a conditional set encoder (ragged sequences).

Sharding: data-parallel over the task/batch dim B=64 across 8 NeuronCores
(8 tasks per core); all weights replicated.

v2 design notes (vs the v1 baseline):
- Host-side prep: the NaN row mask is computed on host; valid context rows
  are compacted (~70% survive) and zero-padded to NKC*128, and C / X_C / X_D
  are pre-transposed to feature-major layout. This removes all on-device
  transposes / NaN handling and cuts MLP + V + attention key work by ~25%.
- Multiplicative masking: exp(logit + mask) == exp(logit) * valid, so
  validity is folded into the V tiles (V rows and the softmax-ones column
  are zeroed for pad keys). The exp activation then needs no per-key-chunk
  bias, letting one ACT instruction cover N=1024 (a 2-bank PSUM span) --
  the v1 per-512-tile exp overhead was the pipeline bottleneck (ACT ~1.44us
  per kc vs PE ~0.85us, leaving the PE idle enough that the HAM clock gate
  kept it at 1.2 GHz through the whole attention phase).
- The attention inner loop emits logits for chunk kc+1 before the AV matmuls
  of chunk kc (double-buffered 2-bank logit tiles) so the PE runs ahead of
  the exp stream.
- MLP/V-projection path runs in bf16 (weights + activations); the logits
  path (x, q, k) and all block-level tensors stay fp32r for accuracy.

On-chip layout: activations are feature-major ("T layout", [feat, row]).
Attention per head pair in [k, q] layout:
  PL[k, 0:512]   = k_even . q_even   (K=64)
  PL[k, 512:1024]= k_odd  . q_odd
  E = exp(SCALE * PL)                (one ACT instr, no bias)
  orw[d|s, q]   += (V_ext[k, d|valid])^T @ E    (ones col * valid -> sums)
then o is normalized by s via a K=8 broadcast matmul + divide.
"""

import math
from contextlib import ExitStack

import numpy as np
import ml_dtypes

import concourse.bacc as bacc
import concourse.bass as bass  # noqa: F401  (bass types via bacc)
import concourse.tile as tile
from concourse import mybir
from concourse.alu_op_type import AluOpType
from concourse.bass_utils import run_bass_kernel_spmd
from concourse.masks import make_identity

# Problem dims (hardcoded per spec)
B, NC, ND = 64, 1024, 1024
DX, DY, DH = 64, 64, 512
NH, HD = 8, 64
NCORES = 8
TPC = B // NCORES  # tasks per core

F32 = mybir.dt.float32
F32R = mybir.dt.float32r
BF16 = mybir.dt.bfloat16
I32 = mybir.dt.int32
AF = mybir.ActivationFunctionType

SCALE = 1.0 / math.sqrt(DH)

MM_DT = F32R  # fp32 matmul streaming dtype: float32r is full-rate at N>=256

CH = 4  # 128-feature chunks of DH


def _col_chunks(n, w=512):
    out = []
    s = 0
    while s < n:
        out.append(slice(s, min(s + w, n)))
        s += w
    return out


def build_nc(n_tasks, nkc, mm_dt=MM_DT):
    NK = nkc * 128
    nc = bacc.Bacc(None, target_bir_lowering=False, dynamic_dma_scratch_size=256)

    def mm(out, lhsT, rhs, **kw):
        nc.tensor.matmul(out, lhsT.bitcast(mm_dt), rhs.bitcast(mm_dt), **kw)

    def r(ap):
        # write-side cast: producers of fp32r matmul inputs must round to the
        # matmul dtype (walrus verifies fp32r producer/consumer pairing)
        return ap.bitcast(mm_dt)

    # ---- DRAM I/O (host pre-compacted / pre-transposed) ----
    CzT_d = nc.dram_tensor("CzT", [n_tasks, DX + DY, NK], BF16, kind="ExternalInput")
    XCT_d = nc.dram_tensor("XCT", [n_tasks, DX, NK], F32, kind="ExternalInput")
    XDT_d = nc.dram_tensor("XDT", [n_tasks, DX, ND], F32, kind="ExternalInput")
    VALR_d = nc.dram_tensor("VALR", [n_tasks, 1, NK], F32, kind="ExternalInput")
    W0_d = nc.dram_tensor("W0", [DX + DY, DH], BF16, kind="ExternalInput")
    b0_d = nc.dram_tensor("b0", [DH], F32, kind="ExternalInput")
    Wm_d = nc.dram_tensor("Wm", [2, DH, DH], BF16, kind="ExternalInput")
    bm_d = nc.dram_tensor("bm", [2, DH], F32, kind="ExternalInput")
    Wl_d = nc.dram_tensor("Wl", [DH, DH], BF16, kind="ExternalInput")
    bl_d = nc.dram_tensor("bl", [DH], F32, kind="ExternalInput")
    temb_d = nc.dram_tensor("task_emb", [DH], F32, kind="ExternalInput")
    Wq0_d = nc.dram_tensor("Wq0", [DX, DH], F32, kind="ExternalInput")
    Wk0_d = nc.dram_tensor("Wk0", [DX, DH], F32, kind="ExternalInput")
    Wv0_d = nc.dram_tensor("Wv0", [DH, DH], BF16, kind="ExternalInput")
    Wo0_d = nc.dram_tensor("Wo0", [DH, DH], F32, kind="ExternalInput")
    bo0_d = nc.dram_tensor("bo0", [DH], F32, kind="ExternalInput")
    Wq1_d = nc.dram_tensor("Wq1", [DH, DH], F32, kind="ExternalInput")
    Wk1_d = nc.dram_tensor("Wk1", [DX, DH], F32, kind="ExternalInput")
    Wv1_d = nc.dram_tensor("Wv1", [DH, DH], BF16, kind="ExternalInput")
    Wo1_d = nc.dram_tensor("Wo1", [DH, DH], F32, kind="ExternalInput")
    bo1_d = nc.dram_tensor("bo1", [DH], F32, kind="ExternalInput")
    OUT_d = nc.dram_tensor("OUT", [n_tasks, ND, DH], F32, kind="ExternalOutput")

    kchunks = _col_chunks(NK)
    dchunks = _col_chunks(ND)

    with tile.TileContext(nc) as tc, ExitStack() as ctx, \
            nc.allow_low_precision(reason="bf16 mlp/value path + fp32r rounding are intentional"):
        wp = ctx.enter_context(tc.tile_pool(name="wp", bufs=1))
        czp = ctx.enter_context(tc.tile_pool(name="czp", bufs=2))
        xp = ctx.enter_context(tc.tile_pool(name="xp", bufs=2))
        hp = ctx.enter_context(tc.tile_pool(name="hp", bufs=2))
        dp = ctx.enter_context(tc.tile_pool(name="dp", bufs=2))
        vexp = ctx.enter_context(tc.tile_pool(name="vexp", bufs=2 * nkc))
        vbsp = ctx.enter_context(tc.tile_pool(name="vbsp", bufs=2))
        bigp = ctx.enter_context(tc.tile_pool(name="bigp", bufs=3))
        ep = ctx.enter_context(tc.tile_pool(name="ep", bufs=3))
        sp = ctx.enter_context(tc.tile_pool(name="sp", bufs=1))
        stp = ctx.enter_context(tc.tile_pool(name="stp", bufs=2))
        ftp = ctx.enter_context(tc.tile_pool(name="ftp", bufs=2))
        urp = ctx.enter_context(tc.tile_pool(name="urp", bufs=2))
        # PSUM: logits double-buffer 2x2 banks + AV accumulators 2 + general 2
        plp = ctx.enter_context(tc.tile_pool(name="plp", bufs=2, space="PSUM"))
        avp = ctx.enter_context(tc.tile_pool(name="avp", bufs=2, space="PSUM"))
        fillp = ctx.enter_context(tc.tile_pool(name="fillp", bufs=2, space="PSUM"))

        def psum():
            return fillp.tile([128, 512], F32, name="ps", tag="ps")

        # ---- constants & weights (loaded once, reused by all tasks) ----
        ident = wp.tile([128, 128], F32, name="ident")
        make_identity(nc, ident)

        ones1f = wp.tile([1, 128], F32, name="ones1f")
        nc.vector.memset(ones1f, 1.0)
        ones1 = wp.tile([1, 128], F32, name="ones1")
        nc.vector.tensor_copy(r(ones1), ones1f)
        ones512f = wp.tile([1, 512], F32, name="ones512f")
        nc.vector.memset(ones512f, 1.0)
        ones512 = wp.tile([1, 512], F32, name="ones512")
        nc.vector.tensor_copy(r(ones512), ones512f)

        # b8c[c][k, p] = 1 iff k == 2c + p//64 : broadcasts S rows (2c,2c+1)
        # over the 128 partitions of feature-chunk c.
        bdiff = wp.tile([8, 128], I32, name="bdiff")
        nc.gpsimd.iota(bdiff, pattern=[[-1, 2], [0, 64]], channel_multiplier=1)
        bdifff = wp.tile([8, 128], F32, name="bdifff")
        nc.vector.tensor_copy(bdifff, bdiff)
        b8 = []
        for c in range(CH):
            b8c = wp.tile([8, 128], F32, name=f"b8_{c}", tag=f"b8_{c}")
            nc.vector.tensor_scalar(out=r(b8c), in0=bdifff, scalar1=float(2 * c),
                                    scalar2=None, op0=AluOpType.is_equal)
            b8.append(b8c)

        def load_w_chunked(name, src_ap, dt=F32):
            # [DH, DH] weight -> [128, CH, DH]: partition p + chunk c = row c*128+p
            t = wp.tile([128, CH, DH], dt, name=name, tag=name)
            src = src_ap.rearrange("(c p) n -> p c n", p=128)
            if dt == F32:
                nc.sync.dma_start(out=r(t), in_=src.bitcast(mm_dt))
            else:
                nc.sync.dma_start(out=t, in_=src)
            return t

        def load_col(name, src_ap):
            # [DH] vector -> [128, CH] column tiles
            t = wp.tile([128, CH], F32, name=name, tag=name)
            nc.sync.dma_start(out=t, in_=src_ap.rearrange("(c p) -> p c", p=128))
            return t

        W0s = wp.tile([128, DH], BF16, name="W0s")
        nc.sync.dma_start(out=W0s, in_=W0_d[:, :])
        Wm0s = load_w_chunked("Wm0s", Wm_d[0], BF16)
        Wm1s = load_w_chunked("Wm1s", Wm_d[1], BF16)
        Wls = load_w_chunked("Wls", Wl_d[:, :], BF16)
        Wv0s = load_w_chunked("Wv0s", Wv0_d[:, :], BF16)
        Wv1s = load_w_chunked("Wv1s", Wv1_d[:, :], BF16)
        Wo0s = load_w_chunked("Wo0s", Wo0_d[:, :])
        Wq1s = load_w_chunked("Wq1s", Wq1_d[:, :])
        Wo1s = load_w_chunked("Wo1s", Wo1_d[:, :])

        Wq0s = wp.tile([64, DH], F32, name="Wq0s")
        nc.sync.dma_start(out=r(Wq0s), in_=Wq0_d[:, :].bitcast(mm_dt))
        Wk0s = wp.tile([64, DH], F32, name="Wk0s")
        nc.sync.dma_start(out=r(Wk0s), in_=Wk0_d[:, :].bitcast(mm_dt))
        Wk1s = wp.tile([64, DH], F32, name="Wk1s")
        nc.sync.dma_start(out=r(Wk1s), in_=Wk1_d[:, :].bitcast(mm_dt))

        b0c = load_col("b0c", b0_d[:])
        bm0c = load_col("bm0c", bm_d[0])
        bm1c = load_col("bm1c", bm_d[1])
        blc = load_col("blc", bl_d[:])
        tembc = load_col("tembc", temb_d[:])
        bo0c = load_col("bo0c", bo0_d[:])
        # d-layer bias: bl + task_emb fused
        dbc = wp.tile([128, CH], F32, name="dbc")
        nc.vector.tensor_add(dbc, blc, tembc)
        bo1r = wp.tile([1, DH], F32, name="bo1r")
        nc.sync.dma_start(out=r(bo1r),
                          in_=bo1_d[:].rearrange("(a n) -> a n", a=1).bitcast(mm_dt))

        def proj64(Ws, xT, outT, chunks):
            """outT[feat, m] = (x @ W)^T for 64-dim input x (xT: [64, n])."""
            for c in range(CH):
                for ms in chunks:
                    n = ms.stop - ms.start
                    ps = psum()
                    mm(ps[:, 0:n], Ws[:, c * 128:(c + 1) * 128], xT[:, ms],
                       start=True, stop=True)
                    nc.any.tensor_copy(r(outT[:, c, ms]), ps[:, 0:n])

        def attn_block(qT, kT, ve_list, uT):
            """uT = qT + softmax(scale*q.k)*valid @ v   (all in T layout)."""
            S = sp.tile([8, ND], F32, name="S", tag="S")
            for hp2 in range(NH // 2):
                he, ho = 2 * hp2, 2 * hp2 + 1
                c = hp2  # feature chunk holding this head pair
                for qc in range(2):
                    qs = slice(qc * 512, qc * 512 + 512)
                    orwE = avp.tile([128, 512], F32, name="orwE", tag="orw")
                    orwO = avp.tile([128, 512], F32, name="orwO", tag="orw")
                    E_tiles = [None] * nkc
                    # run logits one kc ahead of the AV consumers
                    for kc in range(nkc + 1):
                        if kc < nkc:
                            ks = slice(kc * 128, kc * 128 + 128)
                            PL = plp.tile([128, 1024], F32, name="PL", tag="PL")
                            mm(PL[:, 0:512], kT[0:64, c, ks], qT[0:64, c, qs],
                               start=True, stop=True)
                            mm(PL[:, 512:1024], kT[64:128, c, ks], qT[64:128, c, qs],
                               start=True, stop=True)
                            E2 = ep.tile([128, 1024], BF16, name="E2", tag="E")
                            nc.scalar.activation(E2, PL, AF.Exp, scale=SCALE)
                            E_tiles[kc] = E2
                        if kc > 0:
                            pk = kc - 1
                            Ep = E_tiles[pk]
                            nc.tensor.matmul(orwE[0:65, :], ve_list[pk][:, he, :],
                                             Ep[:, 0:512],
                                             start=(pk == 0), stop=(pk == nkc - 1))
                            nc.tensor.matmul(orwO[0:65, :], ve_list[pk][:, ho, :],
                                             Ep[:, 512:1024],
                                             start=(pk == 0), stop=(pk == nkc - 1))
                    # stash unnormalized oT and the softmax sums; sums hop
                    # through an SBUF stage row (engine APs can only start at
                    # 32-aligned partitions; DMA can write any partition).
                    nc.vector.tensor_copy(r(uT[0:64, c, qs]), orwE[0:64, :])
                    nc.vector.tensor_copy(r(uT[64:128, c, qs]), orwO[0:64, :])
                    stE = stp.tile([1, 512], F32, name="stE", tag="st")
                    stO = stp.tile([1, 512], F32, name="stO", tag="st")
                    nc.vector.tensor_copy(r(stE), orwE[64:65, :])
                    nc.vector.tensor_copy(r(stO), orwO[64:65, :])
                    nc.sync.dma_start(out=r(S[he:he + 1, qs]), in_=r(stE))
                    nc.sync.dma_start(out=r(S[ho:ho + 1, qs]), in_=r(stO))
            # 1/s on ACT via exp(-ln(s)): DVE reciprocal on an 8-partition
            # tile is ~6.5us (8 cyc/elem iterative divide, 8/128 lanes);
            # two ACT passes are ~2.3us and ln/exp share one table set.
            Sl = sp.tile([8, ND], F32, name="Sl", tag="Sl")
            nc.scalar.activation(Sl, S, AF.Ln)
            Si = sp.tile([8, ND], F32, name="Si", tag="Si")
            nc.scalar.activation(r(Si), Sl, AF.Exp, scale=-1.0)
            # normalize by 1/s (partition-broadcast via K=8 outer product) + resid
            for c in range(CH):
                for mh in range(2):
                    ms = slice(mh * 512, mh * 512 + 512)
                    bc = psum()
                    mm(bc, b8[c], Si[:, ms], start=True, stop=True)
                    nc.vector.tensor_mul(r(uT[:, c, ms]), uT[:, c, ms], bc)
                nc.vector.tensor_add(r(uT[:, c, :]), uT[:, c, :], qT[:, c, :])

        # =================== per-task pipeline ===================
        for t in range(n_tasks):
            # ---- stage A: load pre-transposed inputs ----
            czT = czp.tile([128, NK], BF16, name="czT", tag="czT")
            nc.sync.dma_start(out=czT, in_=CzT_d[t])
            xcT = xp.tile([64, NK], F32, name="xcT", tag="xcT")
            nc.sync.dma_start(out=r(xcT), in_=XCT_d[t].bitcast(mm_dt))
            xdT = xp.tile([64, ND], F32, name="xdT", tag="xdT")
            nc.sync.dma_start(out=r(xdT), in_=XDT_d[t].bitcast(mm_dt))
            valr = xp.tile([1, NK], F32, name="valr", tag="valr")
            nc.sync.dma_start(out=r(valr), in_=VALR_d[t].bitcast(mm_dt))

            # ---- MLP in bf16 T layout ----
            h1 = hp.tile([128, CH, NK], BF16, name="h1", tag="h")
            for c in range(CH):
                for ms in kchunks:
                    n = ms.stop - ms.start
                    ps = psum()
                    nc.tensor.matmul(ps[:, 0:n], W0s[:, c * 128:(c + 1) * 128],
                                     czT[:, ms], start=True, stop=True)
                    nc.scalar.activation(h1[:, c, ms], ps[:, 0:n], AF.Relu,
                                         bias=b0c[:, c:c + 1])
            h2 = hp.tile([128, CH, NK], BF16, name="h2", tag="h")
            for c in range(CH):
                for ms in kchunks:
                    n = ms.stop - ms.start
                    ps = psum()
                    for kcf in range(CH):
                        nc.tensor.matmul(ps[:, 0:n],
                                         Wm0s[:, kcf, c * 128:(c + 1) * 128],
                                         h1[:, kcf, ms],
                                         start=(kcf == 0), stop=(kcf == CH - 1))
                    nc.scalar.activation(h2[:, c, ms], ps[:, 0:n], AF.Relu,
                                         bias=bm0c[:, c:c + 1])
            h3 = hp.tile([128, CH, NK], BF16, name="h3", tag="h")
            for c in range(CH):
                for ms in kchunks:
                    n = ms.stop - ms.start
                    ps = psum()
                    for kcf in range(CH):
                        nc.tensor.matmul(ps[:, 0:n],
                                         Wm1s[:, kcf, c * 128:(c + 1) * 128],
                                         h2[:, kcf, ms],
                                         start=(kcf == 0), stop=(kcf == CH - 1))
                    nc.scalar.activation(h3[:, c, ms], ps[:, 0:n], AF.Relu,
                                         bias=bm1c[:, c:c + 1])
            d_t = dp.tile([128, CH, NK], BF16, name="d_t", tag="d")
            for c in range(CH):
                for ms in kchunks:
                    n = ms.stop - ms.start
                    ps = psum()
                    for kcf in range(CH):
                        nc.tensor.matmul(ps[:, 0:n],
                                         Wls[:, kcf, c * 128:(c + 1) * 128],
                                         h3[:, kcf, ms],
                                         start=(kcf == 0), stop=(kcf == CH - 1))
                    nc.scalar.activation(d_t[:, c, ms], ps[:, 0:n], AF.Identity,
                                         bias=dbc[:, c:c + 1])

            # ---- V projections with validity folded in ----
            # vb[p, j] = valid[kc*128+p] for all j (K=1 broadcast matmul)
            v0e = [None] * nkc
            v1e = [None] * nkc
            for kc in range(nkc):
                ks = slice(kc * 128, kc * 128 + 128)
                vbps = psum()
                mm(vbps, valr[:, ks], ones512, start=True, stop=True)
                vbs = vbsp.tile([128, 512], F32, name="vbs", tag="vbs")
                nc.vector.tensor_copy(vbs, vbps)
                for vlist, Wvs in ((v0e, Wv0s), (v1e, Wv1s)):
                    ps = psum()
                    for kcf in range(CH):
                        nc.tensor.matmul(ps, d_t[:, kcf, ks], Wvs[:, kcf, :],
                                         start=(kcf == 0), stop=(kcf == CH - 1))
                    ve = vexp.tile([128, NH, HD + 1], BF16, name="ve", tag="ve")
                    # V rows scaled by validity (zero for pad keys)
                    nc.vector.tensor_tensor(
                        out=ve[:, :, 0:HD],
                        in0=ps.rearrange("p (h e) -> p h e", h=NH),
                        in1=vbs.rearrange("p (h e) -> p h e", h=NH),
                        op=AluOpType.mult)
                    # softmax-ones column = validity
                    nc.vector.tensor_copy(ve[:, :, HD:HD + 1],
                                          vbs[:, 0:NH].rearrange("p (h a) -> p h a", a=1))
                    vlist[kc] = ve

            # ---- projections for block 0 ----
            k0T = bigp.tile([128, CH, NK], F32, name="k0T", tag="big")
            proj64(Wk0s, xcT, k0T, kchunks)
            q0T = bigp.tile([128, CH, ND], F32, name="q0T", tag="big")
            proj64(Wq0s, xdT, q0T, dchunks)

            # ---- block 0 attention (+ residual into uT) ----
            uT = bigp.tile([128, CH, ND], F32, name="uT", tag="big")
            attn_block(q0T, k0T, v0e, uT)

            # k1 projection (placed here: fills PE while block-0 ACT drains)
            k1T = bigp.tile([128, CH, NK], F32, name="k1T", tag="big")
            proj64(Wk1s, xcT, k1T, kchunks)

            # ---- block 0 FFN: u2 = u + relu(u @ Wo0 + bo0) ----
            u2T = bigp.tile([128, CH, ND], F32, name="u2T", tag="big")
            for c in range(CH):
                for mh in range(2):
                    ms = slice(mh * 512, mh * 512 + 512)
                    ps = psum()
                    for kcf in range(CH):
                        mm(ps, Wo0s[:, kcf, c * 128:(c + 1) * 128],
                           uT[:, kcf, ms], start=(kcf == 0), stop=(kcf == CH - 1))
                    ft = ftp.tile([128, 512], F32, name="ft", tag="ft")
                    nc.scalar.activation(ft, ps, AF.Relu, bias=bo0c[:, c:c + 1])
                    nc.vector.tensor_add(r(u2T[:, c, ms]), uT[:, c, ms], ft)

            # ---- q1 projection ----
            q1T = bigp.tile([128, CH, ND], F32, name="q1T", tag="big")
            for c in range(CH):
                for mh in range(2):
                    ms = slice(mh * 512, mh * 512 + 512)
                    ps = psum()
                    for kcf in range(CH):
                        mm(ps, Wq1s[:, kcf, c * 128:(c + 1) * 128],
                           u2T[:, kcf, ms], start=(kcf == 0), stop=(kcf == CH - 1))
                    nc.any.tensor_copy(r(q1T[:, c, ms]), ps)

            # ---- block 1 attention ----
            uT1 = bigp.tile([128, CH, ND], F32, name="uT1", tag="big")
            attn_block(q1T, k1T, v1e, uT1)

            # ---- block 1 FFN in row-major + output ----
            for j in range(ND // 128):
                js = slice(j * 128, j * 128 + 128)
                fp = psum()
                # bias via K=1 outer product, then accumulate u @ Wo1
                mm(fp, ones1, bo1r, start=True, stop=False)
                for kcf in range(CH):
                    mm(fp, uT1[:, kcf, js], Wo1s[:, kcf, :],
                       start=False, stop=(kcf == CH - 1))
                fr = ftp.tile([128, DH], F32, name="fr", tag="ft")
                nc.scalar.activation(fr, fp, AF.Relu)
                ur = urp.tile([128, DH], F32, name="ur", tag="ur")
                for c in range(CH):
                    cs = slice(c * 128, c * 128 + 128)
                    tp = psum()
                    nc.tensor.transpose(tp[:, 0:128], uT1[:, c, js], ident)
                    nc.vector.tensor_add(ur[:, cs], fr[:, cs], tp[:, 0:128])
                nc.sync.dma_start(out=OUT_d[t, js, :], in_=ur)

    nc.compile()
    return nc


_NC_CACHE = {}


def _get_nc(nkc):
    key = (TPC, nkc, MM_DT)
    if key not in _NC_CACHE:
        _NC_CACHE[key] = build_nc(TPC, nkc, MM_DT)
    return _NC_CACHE[key]


def _as_f32(x):
    return np.ascontiguousarray(np.asarray(x, dtype=np.float32))


def _as_bf16(x):
    return np.ascontiguousarray(np.asarray(x, dtype=np.float32).astype(ml_dtypes.bfloat16))


def _host_prep(C, XC, XD):
    """Compact valid context rows, zero NaNs, pre-transpose; pad to NKC*128."""
    nb = C.shape[0]
    mask = np.isnan(C[:, :, -1])  # [B, NC] True = padded row
    nvs = (~mask).sum(axis=1)
    nkc = max(1, int(-(-int(nvs.max()) // 128)))
    nk = nkc * 128
    czT = np.zeros((nb, DX + DY, nk), dtype=ml_dtypes.bfloat16)
    xcT = np.zeros((nb, DX, nk), dtype=np.float32)
    valr = np.zeros((nb, 1, nk), dtype=np.float32)
    for t in range(nb):
        idx = np.flatnonzero(~mask[t])
        nv = idx.size
        czT[t, :, :nv] = C[t, idx].T.astype(ml_dtypes.bfloat16)
        xcT[t, :, :nv] = XC[t, idx].T
        valr[t, 0, :nv] = 1.0
    xdT = np.ascontiguousarray(XD.transpose(0, 2, 1))
    return czT, xcT, xdT, valr, nkc


def run(inputs, trace=False, **kw):
    C = _as_f32(inputs["C"])
    XC = _as_f32(inputs["X_C"])
    XD = _as_f32(inputs["X_D"])
    czT, xcT, xdT, valr, nkc = _host_prep(C, XC, XD)
    nc = _get_nc(nkc)
    f32_names = ["b0", "bm", "bl", "task_emb", "Wq0", "Wk0", "Wo0", "bo0",
                 "Wq1", "Wk1", "Wo1", "bo1"]
    bf16_names = ["W0", "Wm", "Wl", "Wv0", "Wv1"]
    weights = {k: _as_f32(inputs[k]) for k in f32_names}
    weights.update({k: _as_bf16(inputs[k]) for k in bf16_names})
    in_maps = []
    for i in range(NCORES):
        s = slice(i * TPC, (i + 1) * TPC)
        m = dict(weights)
        m["CzT"] = czT[s]
        m["XCT"] = xcT[s]
        m["XDT"] = xdT[s]
        m["VALR"] = valr[s]
        in_maps.append(m)
    res = run_bass_kernel_spmd(nc, in_maps, core_ids=list(range(NCORES)),
                               trace=trace, **kw)
    out = np.concatenate([res.results[i]["OUT"] for i in range(NCORES)], axis=0)
    return out, res


def kernel(**inputs) -> np.ndarray:
    out, _ = run(inputs, trace=False)
    return out


# revision 8
# speedup vs baseline: 1.7727x; 1.0427x over previous
"""Trainium2 Bass kernel for a conditional set encoder (ragged sequences).

Sharding: data-parallel over the task/batch dim B=64 across 8 NeuronCores
(8 tasks per core); all weights replicated.

v2 design notes (vs the v1 baseline):
- Host-side prep: the NaN row mask is computed on host; valid context rows
  are compacted (~70% survive) and zero-padded to NKC*128, and C / X_C / X_D
  are pre-transposed to feature-major layout. This removes all on-device
  transposes / NaN handling and cuts MLP + V + attention key work by ~25%.
- Multiplicative masking: exp(logit + mask) == exp(logit) * valid, so
  validity is folded into the V tiles (V rows and the softmax-ones column
  are zeroed for pad keys). The exp activation then needs no per-key-chunk
  bias, letting one ACT instruction cover N=1024 (a 2-bank PSUM span) --
  the v1 per-512-tile exp overhead was the pipeline bottleneck (ACT ~1.44us
  per kc vs PE ~0.85us, leaving the PE idle enough that the HAM clock gate
  kept it at 1.2 GHz through the whole attention phase).
- The attention inner loop emits logits for chunk kc+1 before the AV matmuls
  of chunk kc (double-buffered 2-bank logit tiles) so the PE runs ahead of
  the exp stream.
- MLP/V-projection path runs in bf16 (weights + activations); the logits
  path (x, q, k) and all block-level tensors stay fp32r for accuracy.

On-chip layout: activations are feature-major ("T layout", [feat, row]).
Attention per head pair in [k, q] layout:
  PL[k, 0:512]   = k_even . q_even   (K=64)
  PL[k, 512:1024]= k_odd  . q_odd
  E = exp(SCALE * PL)                (one ACT instr, no bias)
  orw[d|s, q]   += (V_ext[k, d|valid])^T @ E    (ones col * valid -> sums)
then o is normalized by s via a K=8 broadcast matmul + divide.
"""

import math
from contextlib import ExitStack

import numpy as np
import ml_dtypes

import concourse.bacc as bacc
import concourse.bass as bass  # noqa: F401  (bass types via bacc)
import concourse.tile as tile
from concourse import mybir
from concourse.alu_op_type import AluOpType
from concourse.bass_utils import run_bass_kernel_spmd
from concourse.masks import make_identity

# Problem dims (hardcoded per spec)
B, NC, ND = 64, 1024, 1024
DX, DY, DH = 64, 64, 512
NH, HD = 8, 64
NCORES = 8
TPC = B // NCORES  # tasks per core

F32 = mybir.dt.float32
F32R = mybir.dt.float32r
BF16 = mybir.dt.bfloat16
I32 = mybir.dt.int32
AF = mybir.ActivationFunctionType

SCALE = 1.0 / math.sqrt(DH)

MM_DT = F32R  # fp32 matmul streaming dtype: float32r is full-rate at N>=256

CH = 4  # 128-feature chunks of DH


def _col_chunks(n, w=512):
    out = []
    s = 0
    while s < n:
        out.append(slice(s, min(s + w, n)))
        s += w
    return out


def build_nc(n_tasks, nkc, mm_dt=MM_DT):
    NK = nkc * 128
    nc = bacc.Bacc(None, target_bir_lowering=False, dynamic_dma_scratch_size=256)

    def mm(out, lhsT, rhs, **kw):
        nc.tensor.matmul(out, lhsT.bitcast(mm_dt), rhs.bitcast(mm_dt), **kw)

    def r(ap):
        # write-side cast: producers of fp32r matmul inputs must round to the
        # matmul dtype (walrus verifies fp32r producer/consumer pairing)
        return ap.bitcast(mm_dt)

    # ---- DRAM I/O (host pre-compacted / pre-transposed) ----
    CzT_d = nc.dram_tensor("CzT", [n_tasks, DX + DY, NK], BF16, kind="ExternalInput")
    XCT_d = nc.dram_tensor("XCT", [n_tasks, DX, NK], F32, kind="ExternalInput")
    XDT_d = nc.dram_tensor("XDT", [n_tasks, DX, ND], F32, kind="ExternalInput")
    VALR_d = nc.dram_tensor("VALR", [n_tasks, 1, NK], F32, kind="ExternalInput")
    W0_d = nc.dram_tensor("W0", [DX + DY, DH], BF16, kind="ExternalInput")
    b0_d = nc.dram_tensor("b0", [DH], F32, kind="ExternalInput")
    Wm_d = nc.dram_tensor("Wm", [2, DH, DH], BF16, kind="ExternalInput")
    bm_d = nc.dram_tensor("bm", [2, DH], F32, kind="ExternalInput")
    Wl_d = nc.dram_tensor("Wl", [DH, DH], BF16, kind="ExternalInput")
    bl_d = nc.dram_tensor("bl", [DH], F32, kind="ExternalInput")
    temb_d = nc.dram_tensor("task_emb", [DH], F32, kind="ExternalInput")
    Wq0_d = nc.dram_tensor("Wq0", [DX, DH], F32, kind="ExternalInput")
    Wk0_d = nc.dram_tensor("Wk0", [DX, DH], F32, kind="ExternalInput")
    Wv0_d = nc.dram_tensor("Wv0", [DH, DH], BF16, kind="ExternalInput")
    Wo0_d = nc.dram_tensor("Wo0", [DH, DH], F32, kind="ExternalInput")
    bo0_d = nc.dram_tensor("bo0", [DH], F32, kind="ExternalInput")
    Wq1_d = nc.dram_tensor("Wq1", [DH, DH], F32, kind="ExternalInput")
    Wk1_d = nc.dram_tensor("Wk1", [DX, DH], F32, kind="ExternalInput")
    Wv1_d = nc.dram_tensor("Wv1", [DH, DH], BF16, kind="ExternalInput")
    Wo1_d = nc.dram_tensor("Wo1", [DH, DH], F32, kind="ExternalInput")
    bo1_d = nc.dram_tensor("bo1", [DH], F32, kind="ExternalInput")
    OUT_d = nc.dram_tensor("OUT", [n_tasks, ND, DH], F32, kind="ExternalOutput")

    kchunks = _col_chunks(NK)
    dchunks = _col_chunks(ND)

    with tile.TileContext(nc) as tc, ExitStack() as ctx, \
            nc.allow_low_precision(reason="bf16 mlp/value path + fp32r rounding are intentional"):
        wp = ctx.enter_context(tc.tile_pool(name="wp", bufs=1))
        czp = ctx.enter_context(tc.tile_pool(name="czp", bufs=2))
        xp = ctx.enter_context(tc.tile_pool(name="xp", bufs=2))
        hp = ctx.enter_context(tc.tile_pool(name="hp", bufs=2))
        dp = ctx.enter_context(tc.tile_pool(name="dp", bufs=2))
        vexp = ctx.enter_context(tc.tile_pool(name="vexp", bufs=4 * nkc))
        vbsp = ctx.enter_context(tc.tile_pool(name="vbsp", bufs=2))
        bigp = ctx.enter_context(tc.tile_pool(name="bigp", bufs=3))
        ep = ctx.enter_context(tc.tile_pool(name="ep", bufs=3))
        sp = ctx.enter_context(tc.tile_pool(name="sp", bufs=1))
        stp = ctx.enter_context(tc.tile_pool(name="stp", bufs=2))
        ftp = ctx.enter_context(tc.tile_pool(name="ftp", bufs=2))
        urp = ctx.enter_context(tc.tile_pool(name="urp", bufs=2))
        # PSUM: logits double-buffer 2x2 banks + AV accumulators 2 + general 2
        plp = ctx.enter_context(tc.tile_pool(name="plp", bufs=2, space="PSUM"))
        avp = ctx.enter_context(tc.tile_pool(name="avp", bufs=2, space="PSUM"))
        fillp = ctx.enter_context(tc.tile_pool(name="fillp", bufs=2, space="PSUM"))

        def psum():
            return fillp.tile([128, 512], F32, name="ps", tag="ps")

        # ---- constants & weights (loaded once, reused by all tasks) ----
        ident = wp.tile([128, 128], F32, name="ident")
        make_identity(nc, ident)

        ones1f = wp.tile([1, 128], F32, name="ones1f")
        nc.vector.memset(ones1f, 1.0)
        ones1 = wp.tile([1, 128], F32, name="ones1")
        nc.vector.tensor_copy(r(ones1), ones1f)
        ones512f = wp.tile([1, 512], F32, name="ones512f")
        nc.vector.memset(ones512f, 1.0)
        ones512 = wp.tile([1, 512], F32, name="ones512")
        nc.vector.tensor_copy(r(ones512), ones512f)

        # b8c[c][k, p] = 1 iff k == 2c + p//64 : broadcasts S rows (2c,2c+1)
        # over the 128 partitions of feature-chunk c.
        bdiff = wp.tile([8, 128], I32, name="bdiff")
        nc.gpsimd.iota(bdiff, pattern=[[-1, 2], [0, 64]], channel_multiplier=1)
        bdifff = wp.tile([8, 128], F32, name="bdifff")
        nc.vector.tensor_copy(bdifff, bdiff)
        b8 = []
        for c in range(CH):
            b8c = wp.tile([8, 128], F32, name=f"b8_{c}", tag=f"b8_{c}")
            nc.vector.tensor_scalar(out=r(b8c), in0=bdifff, scalar1=float(2 * c),
                                    scalar2=None, op0=AluOpType.is_equal)
            b8.append(b8c)

        def load_w_chunked(name, src_ap, dt=F32):
            # [DH, DH] weight -> [128, CH, DH]: partition p + chunk c = row c*128+p
            t = wp.tile([128, CH, DH], dt, name=name, tag=name)
            src = src_ap.rearrange("(c p) n -> p c n", p=128)
            if dt == F32:
                nc.sync.dma_start(out=r(t), in_=src.bitcast(mm_dt))
            else:
                nc.sync.dma_start(out=t, in_=src)
            return t

        def load_col(name, src_ap):
            # [DH] vector -> [128, CH] column tiles
            t = wp.tile([128, CH], F32, name=name, tag=name)
            nc.sync.dma_start(out=t, in_=src_ap.rearrange("(c p) -> p c", p=128))
            return t

        W0s = wp.tile([128, DH], BF16, name="W0s")
        nc.sync.dma_start(out=W0s, in_=W0_d[:, :])
        Wm0s = load_w_chunked("Wm0s", Wm_d[0], BF16)
        Wm1s = load_w_chunked("Wm1s", Wm_d[1], BF16)
        Wls = load_w_chunked("Wls", Wl_d[:, :], BF16)
        Wv0s = load_w_chunked("Wv0s", Wv0_d[:, :], BF16)
        Wv1s = load_w_chunked("Wv1s", Wv1_d[:, :], BF16)
        Wo0s = load_w_chunked("Wo0s", Wo0_d[:, :])
        Wq1s = load_w_chunked("Wq1s", Wq1_d[:, :])
        Wo1s = load_w_chunked("Wo1s", Wo1_d[:, :])

        Wq0s = wp.tile([64, DH], F32, name="Wq0s")
        nc.sync.dma_start(out=r(Wq0s), in_=Wq0_d[:, :].bitcast(mm_dt))
        Wk0s = wp.tile([64, DH], F32, name="Wk0s")
        nc.sync.dma_start(out=r(Wk0s), in_=Wk0_d[:, :].bitcast(mm_dt))
        Wk1s = wp.tile([64, DH], F32, name="Wk1s")
        nc.sync.dma_start(out=r(Wk1s), in_=Wk1_d[:, :].bitcast(mm_dt))

        b0c = load_col("b0c", b0_d[:])
        bm0c = load_col("bm0c", bm_d[0])
        bm1c = load_col("bm1c", bm_d[1])
        blc = load_col("blc", bl_d[:])
        tembc = load_col("tembc", temb_d[:])
        bo0c = load_col("bo0c", bo0_d[:])
        # d-layer bias: bl + task_emb fused
        dbc = wp.tile([128, CH], F32, name="dbc")
        nc.vector.tensor_add(dbc, blc, tembc)
        bo1r = wp.tile([1, DH], F32, name="bo1r")
        nc.sync.dma_start(out=r(bo1r),
                          in_=bo1_d[:].rearrange("(a n) -> a n", a=1).bitcast(mm_dt))

        def proj64(Ws, xT, outT, chunks):
            """outT[feat, m] = (x @ W)^T for 64-dim input x (xT: [64, n])."""
            for c in range(CH):
                for ms in chunks:
                    n = ms.stop - ms.start
                    ps = psum()
                    mm(ps[:, 0:n], Ws[:, c * 128:(c + 1) * 128], xT[:, ms],
                       start=True, stop=True)
                    nc.any.tensor_copy(r(outT[:, c, ms]), ps[:, 0:n])

        def pull(filler):
            if filler is not None:
                try:
                    next(filler)
                except StopIteration:
                    pass

        def attn_block(qT, kT, ve_list, uT, filler=None):
            """uT = qT + softmax(scale*q.k)*valid @ v   (all in T layout).

            filler: generator of next-task prep work; one chunk is pulled per
            key-chunk iteration to keep the PE dense while ACT streams exps.
            """
            S = sp.tile([8, ND], F32, name="S", tag="S")
            for hp2 in range(NH // 2):
                he, ho = 2 * hp2, 2 * hp2 + 1
                c = hp2  # feature chunk holding this head pair
                for qc in range(2):
                    qs = slice(qc * 512, qc * 512 + 512)
                    orwE = avp.tile([128, 512], F32, name="orwE", tag="orw")
                    orwO = avp.tile([128, 512], F32, name="orwO", tag="orw")
                    E_tiles = [None] * nkc
                    # run logits one kc ahead of the AV consumers
                    for kc in range(nkc + 1):
                        if kc < nkc:
                            ks = slice(kc * 128, kc * 128 + 128)
                            PL = plp.tile([128, 1024], F32, name="PL", tag="PL")
                            mm(PL[:, 0:512], kT[0:64, c, ks], qT[0:64, c, qs],
                               start=True, stop=True)
                            mm(PL[:, 512:1024], kT[64:128, c, ks], qT[64:128, c, qs],
                               start=True, stop=True)
                            E2 = ep.tile([128, 1024], BF16, name="E2", tag="E")
                            nc.scalar.activation(E2, PL, AF.Exp, scale=SCALE)
                            E_tiles[kc] = E2
                        pull(filler)
                        if kc > 0:
                            pk = kc - 1
                            Ep = E_tiles[pk]
                            nc.tensor.matmul(orwE[0:65, :], ve_list[pk][:, he, :],
                                             Ep[:, 0:512],
                                             start=(pk == 0), stop=(pk == nkc - 1))
                            nc.tensor.matmul(orwO[0:65, :], ve_list[pk][:, ho, :],
                                             Ep[:, 512:1024],
                                             start=(pk == 0), stop=(pk == nkc - 1))
                    # stash unnormalized oT and the softmax sums; sums hop
                    # through an SBUF stage row (engine APs can only start at
                    # 32-aligned partitions; DMA can write any partition).
                    nc.vector.tensor_copy(r(uT[0:64, c, qs]), orwE[0:64, :])
                    nc.vector.tensor_copy(r(uT[64:128, c, qs]), orwO[0:64, :])
                    stE = stp.tile([1, 512], F32, name="stE", tag="st")
                    stO = stp.tile([1, 512], F32, name="stO", tag="st")
                    nc.vector.tensor_copy(r(stE), orwE[64:65, :])
                    nc.vector.tensor_copy(r(stO), orwO[64:65, :])
                    nc.sync.dma_start(out=r(S[he:he + 1, qs]), in_=r(stE))
                    nc.sync.dma_start(out=r(S[ho:ho + 1, qs]), in_=r(stO))
            nc.vector.reciprocal(r(S), S)
            # normalize by 1/s (partition-broadcast via K=8 outer product) + resid
            for c in range(CH):
                for mh in range(2):
                    ms = slice(mh * 512, mh * 512 + 512)
                    bc = psum()
                    mm(bc, b8[c], S[:, ms], start=True, stop=True)
                    nc.vector.tensor_mul(r(uT[:, c, ms]), uT[:, c, ms], bc)
                nc.vector.tensor_add(r(uT[:, c, :]), uT[:, c, :], qT[:, c, :])

        # =================== per-task pipeline ===================
        # prep (loads, MLP, V tiles) for task t+1 is emitted as a generator
        # and pulled chunk-by-chunk inside task t's ACT-bound attention loops
        # so the PE never idles (and the HAM clock gate stays at 2.4 GHz).
        state = [dict() for _ in range(n_tasks)]

        def prep_gen(t):
            st = state[t]
            czT = czp.tile([128, NK], BF16, name="czT", tag="czT")
            nc.sync.dma_start(out=czT, in_=CzT_d[t])
            xcT = xp.tile([64, NK], F32, name="xcT", tag="xcT")
            nc.sync.dma_start(out=r(xcT), in_=XCT_d[t].bitcast(mm_dt))
            xdT = xp.tile([64, ND], F32, name="xdT", tag="xdT")
            nc.sync.dma_start(out=r(xdT), in_=XDT_d[t].bitcast(mm_dt))
            valr = xp.tile([1, NK], F32, name="valr", tag="valr")
            nc.sync.dma_start(out=r(valr), in_=VALR_d[t].bitcast(mm_dt))
            st["xcT"], st["xdT"] = xcT, xdT
            yield

            # ---- MLP in bf16 T layout ----
            h1 = hp.tile([128, CH, NK], BF16, name="h1", tag="h")
            for c in range(CH):
                for ms in kchunks:
                    n = ms.stop - ms.start
                    ps = psum()
                    nc.tensor.matmul(ps[:, 0:n], W0s[:, c * 128:(c + 1) * 128],
                                     czT[:, ms], start=True, stop=True)
                    nc.scalar.activation(h1[:, c, ms], ps[:, 0:n], AF.Relu,
                                         bias=b0c[:, c:c + 1])
                    yield
            for li, (Wxs, bxc) in enumerate(((Wm0s, bm0c), (Wm1s, bm1c))):
                hn = hp.tile([128, CH, NK], BF16, name=f"h{li + 2}", tag="h")
                for c in range(CH):
                    for ms in kchunks:
                        n = ms.stop - ms.start
                        ps = psum()
                        for kcf in range(CH):
                            nc.tensor.matmul(ps[:, 0:n],
                                             Wxs[:, kcf, c * 128:(c + 1) * 128],
                                             h1[:, kcf, ms],
                                             start=(kcf == 0), stop=(kcf == CH - 1))
                        nc.scalar.activation(hn[:, c, ms], ps[:, 0:n], AF.Relu,
                                             bias=bxc[:, c:c + 1])
                        yield
                h1 = hn
            d_t = dp.tile([128, CH, NK], BF16, name="d_t", tag="d")
            for c in range(CH):
                for ms in kchunks:
                    n = ms.stop - ms.start
                    ps = psum()
                    for kcf in range(CH):
                        nc.tensor.matmul(ps[:, 0:n],
                                         Wls[:, kcf, c * 128:(c + 1) * 128],
                                         h1[:, kcf, ms],
                                         start=(kcf == 0), stop=(kcf == CH - 1))
                    nc.scalar.activation(d_t[:, c, ms], ps[:, 0:n], AF.Identity,
                                         bias=dbc[:, c:c + 1])
                    yield

            # ---- V projections with validity folded in ----
            # vb[p, j] = valid[kc*128+p] for all j (K=1 broadcast matmul)
            v0e = [None] * nkc
            v1e = [None] * nkc
            for kc in range(nkc):
                ks = slice(kc * 128, kc * 128 + 128)
                vbps = psum()
                mm(vbps, valr[:, ks], ones512, start=True, stop=True)
                vbs = vbsp.tile([128, 512], F32, name="vbs", tag="vbs")
                nc.vector.tensor_copy(vbs, vbps)
                yield
                for vlist, Wvs in ((v0e, Wv0s), (v1e, Wv1s)):
                    ps = psum()
                    for kcf in range(CH):
                        nc.tensor.matmul(ps, d_t[:, kcf, ks], Wvs[:, kcf, :],
                                         start=(kcf == 0), stop=(kcf == CH - 1))
                    ve = vexp.tile([128, NH, HD + 1], BF16, name="ve", tag="ve")
                    # V rows scaled by validity (zero for pad keys)
                    nc.vector.tensor_tensor(
                        out=ve[:, :, 0:HD],
                        in0=ps.rearrange("p (h e) -> p h e", h=NH),
                        in1=vbs.rearrange("p (h e) -> p h e", h=NH),
                        op=AluOpType.mult)
                    # softmax-ones column = validity
                    nc.vector.tensor_copy(ve[:, :, HD:HD + 1],
                                          vbs[:, 0:NH].rearrange("p (h a) -> p h a", a=1))
                    vlist[kc] = ve
                    yield
            st["v0e"], st["v1e"] = v0e, v1e

        gens = [prep_gen(t) for t in range(n_tasks)]

        for t in range(n_tasks):
            # finish any prep for this task not already pulled as filler
            for _ in gens[t]:
                pass
            st = state[t]
            xcT, xdT = st["xcT"], st["xdT"]
            v0e, v1e = st["v0e"], st["v1e"]
            filler = gens[t + 1] if t + 1 < n_tasks else None

            # ---- projections for block 0 ----
            k0T = bigp.tile([128, CH, NK], F32, name="k0T", tag="big")
            proj64(Wk0s, xcT, k0T, kchunks)
            q0T = bigp.tile([128, CH, ND], F32, name="q0T", tag="big")
            proj64(Wq0s, xdT, q0T, dchunks)

            # ---- block 0 attention (+ residual into uT) ----
            uT = bigp.tile([128, CH, ND], F32, name="uT", tag="big")
            attn_block(q0T, k0T, v0e, uT, filler)

            # k1 projection (placed here: fills PE while block-0 ACT drains)
            k1T = bigp.tile([128, CH, NK], F32, name="k1T", tag="big")
            proj64(Wk1s, xcT, k1T, kchunks)

            # ---- block 0 FFN: u2 = u + relu(u @ Wo0 + bo0) ----
            u2T = bigp.tile([128, CH, ND], F32, name="u2T", tag="big")
            for c in range(CH):
                for mh in range(2):
                    ms = slice(mh * 512, mh * 512 + 512)
                    ps = psum()
                    for kcf in range(CH):
                        mm(ps, Wo0s[:, kcf, c * 128:(c + 1) * 128],
                           uT[:, kcf, ms], start=(kcf == 0), stop=(kcf == CH - 1))
                    ft = ftp.tile([128, 512], F32, name="ft", tag="ft")
                    nc.scalar.activation(ft, ps, AF.Relu, bias=bo0c[:, c:c + 1])
                    nc.vector.tensor_add(r(u2T[:, c, ms]), uT[:, c, ms], ft)

            # ---- q1 projection ----
            q1T = bigp.tile([128, CH, ND], F32, name="q1T", tag="big")
            for c in range(CH):
                for mh in range(2):
                    ms = slice(mh * 512, mh * 512 + 512)
                    ps = psum()
                    for kcf in range(CH):
                        mm(ps, Wq1s[:, kcf, c * 128:(c + 1) * 128],
                           u2T[:, kcf, ms], start=(kcf == 0), stop=(kcf == CH - 1))
                    nc.any.tensor_copy(r(q1T[:, c, ms]), ps)

            # ---- block 1 attention ----
            uT1 = bigp.tile([128, CH, ND], F32, name="uT1", tag="big")
            attn_block(q1T, k1T, v1e, uT1, filler)

            # ---- block 1 FFN in row-major + output ----
            for j in range(ND // 128):
                js = slice(j * 128, j * 128 + 128)
                fp = psum()
                # bias via K=1 outer product, then accumulate u @ Wo1
                mm(fp, ones1, bo1r, start=True, stop=False)
                for kcf in range(CH):
                    mm(fp, uT1[:, kcf, js], Wo1s[:, kcf, :],
                       start=False, stop=(kcf == CH - 1))
                fr = ftp.tile([128, DH], F32, name="fr", tag="ft")
                nc.scalar.activation(fr, fp, AF.Relu)
                ur = urp.tile([128, DH], F32, name="ur", tag="ur")
                for c in range(CH):
                    cs = slice(c * 128, c * 128 + 128)
                    tp = psum()
                    nc.tensor.transpose(tp[:, 0:128], uT1[:, c, js], ident)
                    nc.vector.tensor_add(ur[:, cs], fr[:, cs], tp[:, 0:128])
                nc.sync.dma_start(out=OUT_d[t, js, :], in_=ur)

    nc.compile()
    return nc


_NC_CACHE = {}


def _get_nc(nkc):
    key = (TPC, nkc, MM_DT)
    if key not in _NC_CACHE:
        _NC_CACHE[key] = build_nc(TPC, nkc, MM_DT)
    return _NC_CACHE[key]


def _as_f32(x):
    return np.ascontiguousarray(np.asarray(x, dtype=np.float32))


def _as_bf16(x):
    return np.ascontiguousarray(np.asarray(x, dtype=np.float32).astype(ml_dtypes.bfloat16))


def _host_prep(C, XC, XD):
    """Compact valid context rows, zero NaNs, pre-transpose; pad to NKC*128."""
    nb = C.shape[0]
    mask = np.isnan(C[:, :, -1])  # [B, NC] True = padded row
    nvs = (~mask).sum(axis=1)
    nkc = max(1, int(-(-int(nvs.max()) // 128)))
    nk = nkc * 128
    czT = np.zeros((nb, DX + DY, nk), dtype=ml_dtypes.bfloat16)
    xcT = np.zeros((nb, DX, nk), dtype=np.float32)
    valr = np.zeros((nb, 1, nk), dtype=np.float32)
    for t in range(nb):
        idx = np.flatnonzero(~mask[t])
        nv = idx.size
        czT[t, :, :nv] = C[t, idx].T.astype(ml_dtypes.bfloat16)
        xcT[t, :, :nv] = XC[t, idx].T
        valr[t, 0, :nv] = 1.0
    xdT = np.ascontiguousarray(XD.transpose(0, 2, 1))
    return czT, xcT, xdT, valr, nkc


def run(inputs, trace=False, **kw):
    C = _as_f32(inputs["C"])
    XC = _as_f32(inputs["X_C"])
    XD = _as_f32(inputs["X_D"])
    czT, xcT, xdT, valr, nkc = _host_prep(C, XC, XD)
    nc = _get_nc(nkc)
    f32_names = ["b0", "bm", "bl", "task_emb", "Wq0", "Wk0", "Wo0", "bo0",
                 "Wq1", "Wk1", "Wo1", "bo1"]
    bf16_names = ["W0", "Wm", "Wl", "Wv0", "Wv1"]
    weights = {k: _as_f32(inputs[k]) for k in f32_names}
    weights.update({k: _as_bf16(inputs[k]) for k in bf16_names})
    in_maps = []
    for i in range(NCORES):
        s = slice(i * TPC, (i + 1) * TPC)
        m = dict(weights)
        m["CzT"] = czT[s]
        m["XCT"] = xcT[s]
        m["XDT"] = xdT[s]
        m["VALR"] = valr[s]
        in_maps.append(m)
    res = run_bass_kernel_spmd(nc, in_maps, core_ids=list(range(NCORES)),
                               trace=trace, **kw)
    out = np.concatenate([res.results[i]["OUT"] for i in range(NCORES)], axis=0)
    return out, res


def kernel(**inputs) -> np.ndarray:
    out, _ = run(inputs, trace=False)
    return out
